# revision 1
# baseline (speedup 1.0000x reference)
"""Bahdanau-attention kernel for Trainium2, data-parallel over 8 NeuronCores.

Per core (B_local=8, T=1024, H=1024), for each batch b:
  encT[o,t] = sum_h W_enc[o,h] * x[t,h]        (PE, f32r, x PE-transposed)
  energyT   = tanh(encT + (W_dec h + b_dec + b_enc)[o])   (ScalarE, fused bias)
  scores[t] = sum_o w_score[o] * energyT[o,t]  (PE, w_score stationary)
  weights   = softmax(scores)                  (DVE/ScalarE, 1 partition)
  context   = sum_t weights[t] * x[t,:]        (PE, natural-layout x)
b_score is dropped: softmax is shift-invariant so it cancels in both outputs.
"""

import sys

if "/opt/trn_rl_repo" not in sys.path:
    sys.path.insert(0, "/opt/trn_rl_repo")

import numpy as np

B, T, H = 64, 1024, 1024
NCORES = 8
BL = B // NCORES  # batches per core
P = 128  # partitions
NT = T // P  # t tiles
NH = H // P  # h chunks
NO = H // P  # o tiles
F = 512  # matmul free-dim slice (one PSUM bank of f32)
NS = T // F  # free-dim halves

_CACHE = {}
LAST_RESULT = None


def build(bl=BL):
    import concourse.tile as tile
    from concourse import bacc, mybir
    from concourse.masks import make_identity

    f32 = mybir.dt.float32
    f32r = mybir.dt.float32r
    AF = mybir.ActivationFunctionType
    AX = mybir.AxisListType

    nc = bacc.Bacc("TRN2", target_bir_lowering=False, debug=False, num_devices=NCORES)
    x_d = nc.declare_dram_parameter("spatial_feats", [bl, T, H], f32r, isOutput=False)
    hs_d = nc.declare_dram_parameter("hidden_state", [bl, H], f32r, isOutput=False)
    we_d = nc.declare_dram_parameter("W_enc", [H, H], f32r, isOutput=False)
    be_d = nc.declare_dram_parameter("b_enc", [H], f32, isOutput=False)
    wd_d = nc.declare_dram_parameter("W_dec", [H, H], f32r, isOutput=False)
    bd_d = nc.declare_dram_parameter("b_dec", [H], f32, isOutput=False)
    ws_d = nc.declare_dram_parameter("w_score", [H], f32r, isOutput=False)
    scr_d = nc.declare_dram_parameter("sc_scratch", [bl, T], f32, isOutput=True)
    ctx_d = nc.declare_dram_parameter("out_ctx", [bl, H], f32, isOutput=True)
    wout_d = nc.declare_dram_parameter("out_w", [bl, T], f32, isOutput=True)

    with tile.TileContext(nc) as tc:
        with (
            tc.tile_pool(name="const", bufs=1) as const,
            tc.tile_pool(name="wt", bufs=NH) as wt_pool,
            tc.tile_pool(name="xnat", bufs=2 * NT + 4) as xnat_pool,
            tc.tile_pool(name="xT", bufs=NH) as xT_pool,
            tc.tile_pool(name="eT", bufs=6) as eT_pool,
            tc.tile_pool(name="rows", bufs=1) as rows,
            tc.tile_pool(name="wdtmp", bufs=4) as wdtmp,
            tc.tile_pool(name="urows", bufs=2) as urows,
            tc.tile_pool(name="small", bufs=12) as small,
            tc.tile_pool(name="mmps", bufs=2, space="PSUM") as mm_ps,
            tc.tile_pool(name="trps", bufs=3, space="PSUM") as tr_ps,
            tc.tile_pool(name="scps", bufs=1, space="PSUM") as sc_ps,
            tc.tile_pool(name="ctxps", bufs=2, space="PSUM") as ctx_ps,
        ):
            # ---- constants ----
            ident_f = const.tile([P, P], f32, tag="identf")
            make_identity(nc, ident_f[:])
            ident = const.tile([P, P], f32r, tag="ident")
            nc.vector.tensor_copy(ident[:], ident_f[:])
            ones8_f = const.tile([1, NT], f32, tag="ones8f")
            nc.gpsimd.memset(ones8_f[:], 1.0)
            ones8 = const.tile([1, NT], f32r, tag="ones8")
            nc.vector.tensor_copy(ones8[:], ones8_f[:])

            def transpose_block(nat_tiles, dst_tile, j):
                """dst_tile <- transpose of column-block j of nat tiles (f32r)."""
                n_rows = len(nat_tiles)
                for half in range((n_rows + 3) // 4):
                    q_cnt = min(4, n_rows - half * 4)
                    ps = tr_ps.tile([P, F], f32r, tag="trps")
                    for q in range(q_cnt):
                        tt = half * 4 + q
                        nc.tensor.transpose(
                            ps[:, q * P : (q + 1) * P],
                            nat_tiles[tt][:, j * P : (j + 1) * P],
                            ident[:],
                        )
                    nc.vector.tensor_copy(
                        dst_tile[:, half * F : half * F + q_cnt * P],
                        ps[:, 0 : q_cnt * P],
                    )

            def transpose_tile(nat, dsts, tt):
                """Column tt of all NH dst tiles <- transpose of one nat tile.
                DMA-paced: only needs this one nat tile resident."""
                for half in range(2):
                    ps = tr_ps.tile([P, F], f32r, tag="trps")
                    for q in range(4):
                        j = half * 4 + q
                        nc.tensor.transpose(
                            ps[:, q * P : (q + 1) * P], nat[:, j * P : (j + 1) * P],
                            ident[:],
                        )
                    for q in range(4):
                        j = half * 4 + q
                        nc.vector.tensor_copy(
                            dsts[j][:, tt * P : (tt + 1) * P], ps[:, q * P : (q + 1) * P]
                        )

            # ---- x(0): DMA first, transpose per-tile as DMA lands ----
            xnat0 = []
            _engs = [nc.sync, nc.scalar, nc.gpsimd]
            for tt in range(NT):
                t = xnat_pool.tile([P, H], f32r, tag="xnat")
                _engs[tt % 3].dma_start(t[:], x_d[0, tt * P : (tt + 1) * P, :])
                xnat0.append(t)
            be_sb = const.tile([P, NH], f32, tag="be")
            nc.gpsimd.dma_start(be_sb[:], be_d[:].rearrange("(c p) -> p c", p=P))
            bd_sb = const.tile([P, NH], f32, tag="bd")
            nc.gpsimd.dma_start(bd_sb[:], bd_d[:].rearrange("(c p) -> p c", p=P))
            bsum = const.tile([P, NH], f32, tag="bsum")
            nc.vector.tensor_add(bsum[:], be_sb[:], bd_sb[:])
            ws_sb = const.tile([P, NH], f32r, tag="ws")
            nc.gpsimd.dma_start(ws_sb[:], ws_d[:].rearrange("(c p) -> p c", p=P))
            hsT = const.tile([P, NH * bl], f32r, tag="hsT")
            for c in range(NH):
                nc.gpsimd.dma_start(
                    hsT[:, c * bl : (c + 1) * bl],
                    hs_d[:, c * P : (c + 1) * P].rearrange("b p -> p b"),
                )

            xT0 = [xT_pool.tile([P, T], f32r, tag="xT", name=f"xT0_{j}") for j in range(NH)]
            for tt in range(NT):
                transpose_tile(xnat0[tt], xT0, tt)

            # ---- per o_tile: W_dec bias, W_enc transposes, and batch-0 enc ----
            # Interleaved so PE has dense real work as soon as DMA lands (warms
            # the HAM clock gate early and hides W processing under enc).
            bias_all = const.tile([P, NO * bl], f32, tag="bias_all")
            wT = [wt_pool.tile([P, H], f32r, tag="wt", name=f"wT_{j}") for j in range(NH)]
            eT0 = {}
            for ot in range(NO):
                wdnat = xnat_pool.tile([P, H], f32r, tag="xnat")
                _engs[(2 * ot) % 3].dma_start(
                    wdnat[:, 0 : H // 2], wd_d[ot * P : (ot + 1) * P, 0 : H // 2]
                )
                _engs[(2 * ot + 1) % 3].dma_start(
                    wdnat[:, H // 2 :], wd_d[ot * P : (ot + 1) * P, H // 2 :]
                )
                wdT = []
                for half in range(2):
                    ps = tr_ps.tile([P, F], f32r, tag="trps")
                    for q in range(4):
                        j = half * 4 + q
                        nc.tensor.transpose(
                            ps[:, q * P : (q + 1) * P], wdnat[:, j * P : (j + 1) * P],
                            ident[:],
                        )
                    w4 = wdtmp.tile([P, F], f32r, tag="wdT")
                    nc.vector.tensor_copy(w4[:], ps[:])
                    wdT.append(w4)
                psd = mm_ps.tile([P, F], f32, tag="mmps")
                for h in range(NH):
                    nc.tensor.matmul(
                        psd[:, 0:bl],
                        wdT[h // 4][:, (h % 4) * P : (h % 4 + 1) * P],
                        hsT[:, h * bl : (h + 1) * bl],
                        start=(h == 0),
                        stop=(h == NH - 1),
                    )
                nc.vector.tensor_scalar_add(
                    bias_all[:, ot * bl : (ot + 1) * bl], psd[:, 0:bl],
                    bsum[:, ot : ot + 1],
                )
                wenat = xnat_pool.tile([P, H], f32r, tag="xnat")
                _engs[(2 * ot + 2) % 3].dma_start(
                    wenat[:, 0 : H // 2], we_d[ot * P : (ot + 1) * P, 0 : H // 2]
                )
                _engs[(2 * ot) % 3].dma_start(
                    wenat[:, H // 2 :], we_d[ot * P : (ot + 1) * P, H // 2 :]
                )
                transpose_tile(wenat, wT, ot)
                for half in range(NS):
                    ps = mm_ps.tile([P, F], f32, tag="mmps")
                    for h in range(NH):
                        nc.tensor.matmul(
                            ps[:],
                            wT[h][:, ot * P : (ot + 1) * P],
                            xT0[h][:, half * F : (half + 1) * F],
                            start=(h == 0),
                            stop=(h == NH - 1),
                        )
                    e = eT_pool.tile([P, F], f32r, tag="eT")
                    nc.scalar.activation(
                        e[:], ps[:], AF.Tanh,
                        bias=bias_all[:, ot * bl : ot * bl + 1], scale=1.0,
                    )
                    eT0[(ot, half)] = e
                if ot == 0:
                    pss0_h = {
                        0: sc_ps.tile([1, F], f32, tag="scps", name="pss0_h0"),
                        1: ctx_ps.tile([1, F], f32, tag="ctxps", name="pss0_h1"),
                    }
                else:
                    for half in range(NS):
                        nc.tensor.matmul(
                            pss0_h[half][:],
                            ws_sb[:, ot - 1 : ot],
                            eT0[(ot - 1, half)][:],
                            start=(ot - 1 == 0),
                            stop=False,
                        )

            # ---- main per-batch pipeline ----
            prev = None
            xnat, xT = xnat0, xT0
            for it in range(bl + 1):
                # stage 1: DMA + transpose x for batch `it` (batch 0 in preamble)
                if 0 < it < bl:
                    xnat = []
                    for tt in range(NT):
                        t = xnat_pool.tile([P, H], f32r, tag="xnat")
                        dma_eng = nc.sync if tt % 2 == 0 else nc.gpsimd
                        dma_eng.dma_start(t[:], x_d[it, tt * P : (tt + 1) * P, :])
                        xnat.append(t)
                    xT = []
                    for j in range(NH):
                        xj = xT_pool.tile([P, T], f32r, tag="xT")
                        transpose_block(xnat, xj, j)
                        xT.append(xj)

                # stage 2: finish batch it-1 (exp-weight transpose + context)
                if it >= 1:
                    b, st = it - 1, prev
                    u_col = st["uT"]
                    ctx_row = rows.tile([1, H], f32, tag="ctxrow")
                    for half in range(NS):
                        pc = ctx_ps.tile([1, F], f32, tag="ctxps")
                        for c in range(NT):
                            nc.tensor.matmul(
                                pc[:],
                                u_col[:, c : c + 1],
                                st["xnat"][c][:, half * F : (half + 1) * F],
                                start=(c == 0),
                                stop=(c == NT - 1),
                            )
                        nc.vector.tensor_scalar_mul(
                            ctx_row[0:1, half * F : (half + 1) * F],
                            pc[0:1, :],
                            st["rz"][0:1, 0:1],
                        )
                    nc.sync.dma_start(ctx_d[b : b + 1, :], ctx_row[:])
                    w_row = rows.tile([1, T], f32, tag="wrow")
                    nc.vector.tensor_scalar_mul(
                        w_row[:], st["u_row"][:], st["rz"][0:1, 0:1]
                    )
                    nc.sync.dma_start(wout_d[b : b + 1, :], w_row[:])

                # stage 3: enc + scores + softmax for batch `it`
                # (scores run one o_tile behind enc, so eT tiles free early
                # and the softmax tail starts right after the last enc group)
                if it < bl:
                    sc_row = rows.tile([1, T], f32, tag="scrow")
                    if it == 0:
                        eT, pss_h = eT0, pss0_h
                    else:
                        eT = {}
                        pss_h = {}
                        for o in range(NO):
                            for half in range(NS):
                                ps = mm_ps.tile([P, F], f32, tag="mmps")
                                for h in range(NH):
                                    nc.tensor.matmul(
                                        ps[:],
                                        wT[h][:, o * P : (o + 1) * P],
                                        xT[h][:, half * F : (half + 1) * F],
                                        start=(h == 0),
                                        stop=(h == NH - 1),
                                    )
                                e = eT_pool.tile([P, F], f32r, tag="eT")
                                nc.scalar.activation(
                                    e[:],
                                    ps[:],
                                    AF.Tanh,
                                    bias=bias_all[:, o * bl + it : o * bl + it + 1],
                                    scale=1.0,
                                )
                                eT[(o, half)] = e
                            if o == 0:
                                pss_h[0] = sc_ps.tile([1, F], f32, tag="scps", name="pss_h0")
                                pss_h[1] = ctx_ps.tile([1, F], f32, tag="ctxps", name="pss_h1")
                            else:
                                for half in range(NS):
                                    nc.tensor.matmul(
                                        pss_h[half][:],
                                        ws_sb[:, o - 1 : o],
                                        eT[(o - 1, half)][:],
                                        start=(o - 1 == 0),
                                        stop=False,
                                    )
                    for half in range(NS):
                        nc.tensor.matmul(
                            pss_h[half][:],
                            ws_sb[:, NO - 1 : NO],
                            eT[(NO - 1, half)][:],
                            start=False,
                            stop=True,
                        )
                        nc.vector.tensor_copy(
                            sc_row[0:1, half * F : (half + 1) * F], pss_h[half][0:1, :]
                        )
                    uT = small.tile([P, NT], f32r, tag="uT")
                    if it == bl - 1:
                        # tail-latency path: columnize exp-weights on PE via
                        # K=1 matmuls against a ones row (no DRAM round-trip)
                        u_rowr = urows.tile([1, T], f32r, tag="urowr")
                        nc.scalar.activation(
                            u_rowr[:], sc_row[:], AF.Exp, bias=0.0, scale=1.0
                        )
                        psw = tr_ps.tile([P, NT * NT], f32, tag="trps")
                        for c in range(NT):
                            nc.tensor.matmul(
                                psw[:, c * NT : (c + 1) * NT],
                                u_rowr[0:1, c * P : (c + 1) * P],
                                ones8[:],
                                start=True,
                                stop=True,
                            )
                        nc.vector.tensor_copy(uT[:], psw[:, 0 : NT * NT : NT])
                    else:
                        nc.sync.dma_start(scr_d[it : it + 1, :], sc_row[0:1, :])
                        scT = small.tile([P, NT], f32, tag="scT")
                        nc.sync.dma_start(
                            scT[:], scr_d[it, :].rearrange("(c p) -> p c", p=P)
                        )
                        nc.scalar.activation(
                            uT[:], scT[:], AF.Exp, bias=0.0, scale=1.0
                        )
                    u_row = urows.tile([1, T], f32, tag="urow")
                    ssum = small.tile([1, 1], f32, tag="ssum")
                    nc.scalar.activation(
                        u_row[:],
                        sc_row[:],
                        AF.Exp,
                        bias=0.0,
                        scale=1.0,
                        accum_out=ssum[:],
                    )
                    rz = small.tile([1, 1], f32, tag="rz")
                    nc.vector.reciprocal(rz[:], ssum[:])
                    prev = {"xnat": xnat, "u_row": u_row, "uT": uT, "rz": rz}

    nc.compile()
    return nc


def _get_nc(bl=BL):
    if bl not in _CACHE:
        _CACHE[bl] = build(bl)
    return _CACHE[bl]


def kernel(**inputs):
    from concourse.bass_utils import run_bass_kernel_spmd

    x = np.ascontiguousarray(np.asarray(inputs["spatial_feats"], dtype=np.float32))
    hs = np.ascontiguousarray(np.asarray(inputs["hidden_state"], dtype=np.float32))
    shared = {
        k: np.ascontiguousarray(np.asarray(inputs[k], dtype=np.float32))
        for k in ("W_enc", "b_enc", "W_dec", "b_dec", "w_score")
    }
    nc = _get_nc()
    in_maps = []
    for i in range(NCORES):
        m = {
            "spatial_feats": x[i * BL : (i + 1) * BL],
            "hidden_state": hs[i * BL : (i + 1) * BL],
        }
        m.update(shared)
        in_maps.append(m)
    res = run_bass_kernel_spmd(nc, in_maps, core_ids=list(range(NCORES)))
    global LAST_RESULT
    LAST_RESULT = res
    ctx = np.concatenate([res.results[i]["out_ctx"] for i in range(NCORES)], axis=0)
    w = np.concatenate([res.results[i]["out_w"] for i in range(NCORES)], axis=0)
    return (ctx, w)

